# revision 1
# baseline (speedup 1.0000x reference)
"""HTSK fuzzy-system kernel for Trainium2 (Bass/Tile), 8-core data-parallel.

Math (per batch row b):
  S     = H/sigma^2 + EPS                          (D,R)
  m     = mean_d(-(X_bd - C_dr)^2 * S_dr)          (B,R)
        = X^2 @ (-S/D) + X @ (2*S*C/D) + K2        (matmul expansion)
  e     = exp(m - max_r m) / sum_r exp(...)        (normalized firing)
  out   = sum_r e_br * G_bro  +  e @ (W2 + 1 b^T)
  G     = X @ Wt,  Wt[d, o*R+r] = W[r*D+d, o]      (B, O*R)  o-major

o-major G columns make the firing-strength multiply read e with a
contiguous innermost r-run (DVE 2x mode) and give the r-reduction tree
strided-outer / flat-write access patterns that also keep 2x mode.

Schedule: all four per-tile prologues (transpose, membership, softmax,
out2) run first — they only need the small DMAs — hiding the ~12 us Wt
stream; then the four G phases run back-to-back (PE dense, Scalar
evicts PSUM, DVE multiplies + reduces, in two o-halves per tile).

Sharding: batch B=4096 split 512 rows per core; weights replicated.
"""
import sys
import types
from contextlib import ExitStack

import numpy as np

sys.path.insert(0, "/opt/trn_rl_repo")

# NTFF profile-hook registry: trn_boot §6 sets it at jax init, concourse
# bass_utils reads it when trace=True. The container's antenv package lacks
# this submodule, so provide it before anything imports jax/concourse.
if "antenv.axon_hooks" not in sys.modules:
    _ah = types.ModuleType("antenv.axon_hooks")
    _ah._hook = None

    def _set_hook(hook):
        _ah._hook = hook

    def _get_hook():
        return _ah._hook

    _ah.set_axon_ntff_profile_hook = _set_hook
    _ah.get_axon_ntff_profile_hook = _get_hook
    sys.modules["antenv.axon_hooks"] = _ah

import ml_dtypes  # noqa: E402
import concourse.bass as bass  # noqa: E402
import concourse.bacc as bacc  # noqa: E402
import concourse.tile as tile  # noqa: E402
from concourse import mybir  # noqa: E402
from concourse import bass_utils  # noqa: E402
from concourse.masks import make_identity  # noqa: E402

H = 0.5
EPS = 1e-8
B, D, R, O = 4096, 256, 128, 64
NCORES = 8
BL = B // NCORES          # 512 batch rows per core
NT = BL // 128            # 4 partition tiles per core
RO = R * O                # 8192
HO = O // 2               # 32 o's per half
HW = HO * R               # 4096 cols per half
F32 = mybir.dt.float32
BF16 = mybir.dt.bfloat16

_CACHE = {}


def _build():
    nc = bacc.Bacc("TRN2", target_bir_lowering=False, debug=False)
    # XC[p, t*D+d] = X[t*128+p, d]  (four 128-row tiles side by side)
    XC = nc.dram_tensor("XC", [128, NT * D], F32, kind="ExternalInput")
    # CB = [A (2x128 cols) | Bm (2x128) | W2p (64)] in bf16
    CB = nc.dram_tensor("CB", [128, 2 * R + 2 * R + O], BF16, kind="ExternalInput")
    K2 = nc.dram_tensor("K2", [1, R], F32, kind="ExternalInput")
    Wt = nc.dram_tensor("Wt", [D, RO], BF16, kind="ExternalInput")
    out = nc.dram_tensor("out", [BL, O], F32, kind="ExternalOutput")

    with tile.TileContext(nc) as tc, ExitStack() as ctx:
        consts = ctx.enter_context(tc.tile_pool(name="consts", bufs=1))
        xtp = ctx.enter_context(tc.tile_pool(name="xtp", bufs=4))
        tlp = ctx.enter_context(tc.tile_pool(name="tlp", bufs=4))
        work = ctx.enter_context(tc.tile_pool(name="work", bufs=2))
        gmp = ctx.enter_context(tc.tile_pool(name="gm", bufs=2))
        treep = ctx.enter_context(tc.tile_pool(name="tree", bufs=2))
        ps_x = ctx.enter_context(tc.tile_pool(name="ps_x", bufs=1, space="PSUM"))
        ps_m = ctx.enter_context(tc.tile_pool(name="ps_m", bufs=1, space="PSUM"))
        ps_e = ctx.enter_context(tc.tile_pool(name="ps_e", bufs=1, space="PSUM"))
        ps_o = ctx.enter_context(tc.tile_pool(name="ps_o", bufs=1, space="PSUM"))
        ps_g = ctx.enter_context(tc.tile_pool(name="ps_g", bufs=2, space="PSUM"))

        # ---- X tiles first on the sync queue (small, unblocks compute),
        # then the other small constants; Wt streams on SWDGE queues in
        # consumption order (quarter-major) ----
        xc_sb = xtp.tile([128, NT * D], F32, tag="xc")
        nc.sync.dma_start(out=xc_sb[:, :], in_=XC[:, :])
        cb_sb = consts.tile([128, 2 * R + 2 * R + O], BF16, tag="cb")
        nc.sync.dma_start(out=cb_sb[:, :], in_=CB[:, :])
        k2_sb = consts.tile([1, R], F32, tag="k2")
        nc.sync.dma_start(out=k2_sb[:, :], in_=K2[:, :])
        xts = [xc_sb[:, t * D:(t + 1) * D] for t in range(NT)]
        a_sb = cb_sb[:, 0:2 * R].rearrange("p (c r) -> p c r", c=2)
        bm_sb = cb_sb[:, 2 * R:4 * R].rearrange("p (c r) -> p c r", c=2)
        w2p_sb = cb_sb[:, 4 * R:4 * R + O]
        identF = consts.tile([128, 128], F32, tag="idf")
        make_identity(nc, identF)
        identB = consts.tile([128, 128], BF16, tag="idb")
        make_identity(nc, identB)
        ones_sb = consts.tile([1, 128], F32, tag="ones")
        nc.vector.memset(ones_sb, 1.0)
        # warm the PE HAM clock-gate during the DMA window: dummy matmuls
        # with no DMA dependencies keep the PE busy >3.4us so the real
        # matmuls run at 2.4 GHz from the start
        wm_ps = ps_g.tile([128, 1024], F32, tag="g", name="warm")
        for _ in range(30):
            nc.tensor.matmul(wm_ps[:, 0:128], lhsT=identF, rhs=identF,
                             start=True, stop=True)
        # gate the big Wt stream behind the last small transfer (k2) so all
        # small transfers win the HBM race (the GpSimd queue is FIFO: this
        # copy waits on the k2 DMA, holding back the Wt pushes behind it)
        gate = consts.tile([1, 4], F32, tag="gate")
        nc.gpsimd.tensor_copy(gate, k2_sb[0:1, 0:4])
        wt_sb = [[None] * 4, [None] * 4]
        for q in range(4):
            for c in range(2):
                t_ = consts.tile([128, 2048], BF16, tag=f"wt{c}{q}",
                                 name=f"wt{c}{q}")
                nc.gpsimd.dma_start(
                    out=t_[:, :], in_=Wt[c * 128:(c + 1) * 128, q * 2048:(q + 1) * 2048]
                )
                wt_sb[c][q] = t_

        # ---- per-tile prologue: transpose, membership, softmax, out2 ----
        xTbs, e_ns, out2s, osbs = {}, {}, {}, {}

        def prologue(t):
            xtT_ps = ps_x.tile([128, D], F32, tag="xtT", name=f"xtT{t}")
            for c in range(2):
                nc.tensor.transpose(
                    xtT_ps[:, c * 128:(c + 1) * 128],
                    xts[t][:, c * 128:(c + 1) * 128],
                    identF,
                )
            x2b = work.tile([128, D], BF16, tag="x2b", name=f"x2b{t}")
            xTb = tlp.tile([128, D], BF16, tag="xTb", name=f"xTb{t}")
            nc.scalar.activation(x2b, xtT_ps, mybir.ActivationFunctionType.Square)
            nc.vector.tensor_copy(xTb, xtT_ps)

            m_ps = ps_m.tile([128, R], F32, tag="m", name=f"m{t}")
            nc.tensor.matmul(m_ps, lhsT=x2b[:, 0:128], rhs=a_sb[:, 0, :],
                             start=True, stop=False)
            nc.tensor.matmul(m_ps, lhsT=x2b[:, 128:256], rhs=a_sb[:, 1, :],
                             start=False, stop=False)
            nc.tensor.matmul(m_ps, lhsT=xTb[:, 0:128], rhs=bm_sb[:, 0, :],
                             start=False, stop=False)
            nc.tensor.matmul(m_ps, lhsT=xTb[:, 128:256], rhs=bm_sb[:, 1, :],
                             start=False, stop=False)
            nc.tensor.matmul(m_ps, lhsT=ones_sb, rhs=k2_sb,
                             start=False, stop=True)

            nmx = work.tile([128, 1], F32, tag="nmx", name=f"nmx{t}")
            nc.vector.reduce_max(nmx, m_ps, axis=mybir.AxisListType.X, negate=True)
            e_bf = work.tile([128, R], BF16, tag="e", name=f"e{t}")
            s_ = work.tile([128, 1], F32, tag="s", name=f"s{t}")
            nc.scalar.activation(e_bf, m_ps, mybir.ActivationFunctionType.Exp,
                                 bias=nmx, scale=1.0, accum_out=s_)
            rs = work.tile([128, 1], F32, tag="rs", name=f"rs{t}")
            nc.vector.reciprocal(rs, s_)
            e_n = tlp.tile([128, R], BF16, tag="en", name=f"en{t}")
            nc.vector.tensor_scalar_mul(e_n, e_bf, rs)

            eT_ps = ps_e.tile([128, 128], BF16, tag="eT", name=f"eT{t}")
            nc.tensor.transpose(eT_ps, e_n, identB)
            eT_sb = work.tile([128, 128], BF16, tag="eTsb", name=f"eTsb{t}")
            nc.scalar.copy(eT_sb, eT_ps)
            out2_ps = ps_o.tile([128, O], F32, tag="out2", name=f"out2{t}")
            nc.tensor.matmul(out2_ps, lhsT=eT_sb, rhs=w2p_sb, start=True, stop=True)
            out2_sb = tlp.tile([128, O], F32, tag="o2sb", name=f"o2sb{t}")
            nc.scalar.copy(out2_sb, out2_ps)

            xTbs[t], e_ns[t], out2s[t] = xTb, e_n, out2_sb
            osbs[t] = work.tile([128, O], F32, tag="osb", name=f"osb{t}")

        # ---- one o-half of G = X @ Wt for tile t: 4x 1024-col PSUM chunks
        # (Scalar evicts), in-place e-multiply, halving tree (DVE) ----
        def g_half(t, h):
            xTb, e_n = xTbs[t], e_ns[t]
            gm = gmp.tile([128, HW], BF16, tag=f"gm{h}", name=f"gm_{t}_{h}")
            for hc in range(4):
                ch = h * 4 + hc
                gt = ps_g.tile([128, 1024], F32, tag="g", name=f"g_{t}_{ch}")
                for c in range(2):
                    for half in range(2):
                        nch = 2 * ch + half
                        q, col = divmod(nch * 512, 2048)
                        nc.tensor.matmul(
                            gt[:, half * 512:(half + 1) * 512],
                            lhsT=xTb[:, c * 128:(c + 1) * 128],
                            rhs=wt_sb[c][q][:, col:col + 512],
                            start=(c == 0), stop=(c == 1),
                        )
                nc.scalar.copy(gm[:, hc * 1024:(hc + 1) * 1024], gt)

            gv = gm.rearrange("p (o r) -> p o r", o=HO)
            ebc = e_n.rearrange("p r -> p () r").broadcast_to((128, HO, R))
            nc.vector.tensor_mul(gv, gv, ebc)

            prev = gm
            r = R
            while r > 2:
                nxt = treep.tile([128, HO * (r // 2)], BF16,
                                 tag=f"tr{h}{r}", name=f"tr_{t}_{h}_{r}")
                pv = prev.rearrange("p (o r) -> p o r", o=HO)
                nv = nxt.rearrange("p (o r) -> p o r", o=HO)
                nc.vector.tensor_add(nv, pv[:, :, 0:r // 2], pv[:, :, r // 2:r])
                prev = nxt
                r //= 2
            pv = prev.rearrange("p (o r) -> p o r", o=HO)
            hsl = slice(h * HO, (h + 1) * HO)
            red = work.tile([128, HO], F32, tag=f"red{h}", name=f"red_{t}_{h}")
            nc.vector.tensor_add(red.rearrange("p o -> p o ()"),
                                 pv[:, :, 0:1], pv[:, :, 1:2])
            nc.vector.tensor_add(osbs[t][:, hsl], red, out2s[t][:, hsl])
            if h == 1:
                nc.sync.dma_start(out=out[t * 128:(t + 1) * 128, :], in_=osbs[t])

        # interleave prologues into the G stream: tile t+1's prologue is
        # emitted between tile t's halves so its softmax overlaps G compute
        prologue(0)
        g_half(0, 0)
        prologue(1)
        g_half(0, 1)
        prologue(2)
        g_half(1, 0)
        prologue(3)
        g_half(1, 1)
        g_half(2, 0)
        g_half(2, 1)
        g_half(3, 0)
        g_half(3, 1)

    nc.finalize()
    return nc


def _get_nc():
    if "nc" not in _CACHE:
        _CACHE["nc"] = _build()
    return _CACHE["nc"]


def _host_prep(centers, sigmas, W, b):
    c64 = centers.astype(np.float64)
    S = (H / sigmas.astype(np.float64) ** 2) + EPS          # (D,R)
    A = (-S / D).astype(ml_dtypes.bfloat16)                  # X^2 coeff
    Bm = (2.0 * S * c64 / D).astype(ml_dtypes.bfloat16)      # X coeff
    K2 = (-(S * c64 * c64).sum(axis=0, keepdims=True) / D).astype(np.float32)
    W1 = W[: D * R].reshape(R, D, O)
    # o-major: Wt[d, o*R + r] = W1[r, d, o]
    Wt = np.ascontiguousarray(W1.transpose(1, 2, 0).reshape(D, RO)).astype(
        ml_dtypes.bfloat16
    )
    W2p = (W[D * R:].astype(np.float64) + b[None, :].astype(np.float64)).astype(
        ml_dtypes.bfloat16
    )
    # CB = [A (c-major) | Bm (c-major) | W2p], all bf16 [128, 576]
    CB = np.concatenate([
        A.reshape(2, 128, R).transpose(1, 0, 2).reshape(128, 2 * R),
        Bm.reshape(2, 128, R).transpose(1, 0, 2).reshape(128, 2 * R),
        W2p,
    ], axis=1)
    return np.ascontiguousarray(CB), K2, Wt


def kernel(X, centers, sigmas, W, b):
    X = np.asarray(X, dtype=np.float32)
    centers = np.asarray(centers, dtype=np.float32)
    sigmas = np.asarray(sigmas, dtype=np.float32)
    W = np.asarray(W, dtype=np.float32)
    b = np.asarray(b, dtype=np.float32)

    CB, K2, Wt = _host_prep(centers, sigmas, W, b)
    nc = _get_nc()
    in_maps = [
        {
            # XC[p, t*D+d] = X_core[t*128+p, d]
            "XC": np.ascontiguousarray(
                X[k * BL:(k + 1) * BL]
                .reshape(NT, 128, D).transpose(1, 0, 2).reshape(128, NT * D)
            ),
            "CB": CB, "K2": K2, "Wt": Wt,
        }
        for k in range(NCORES)
    ]
    res = bass_utils.run_bass_kernel_spmd(nc, in_maps, core_ids=list(range(NCORES)))
    return np.concatenate([res.results[k]["out"] for k in range(NCORES)], axis=0)

